# revision 12
# baseline (speedup 1.0000x reference)
"""GNN NodeModel (message passing) Trainium2 Bass kernel.

Strategy (no collectives needed):
  - Host: sort edges by destination node (col). Shard nodes across the 8
    cores (12500 nodes each), 4 "quarters" of 3125 nodes per core.  Each
    core receives exactly the edges landing in its node range, in sorted
    order, pre-gathered (x[row] materialized per edge) and laid out so the
    whole edge MLP + scatter-mean becomes dense matmuls + a gated scan.
  - Per quarter, each node's edge list is padded to a multiple of 8 slots
    ("blocks").  A column of the shipped edge tensor carries one slot of
    each of the 4 quarters stacked:  catT[4*14, LQ] where the 14 rows are
    [x(7), edge_attr(6), valid(1)].
  - Device:
      mm1: block-diag([W1;b1] x4) [56,128]  ->  P1[128,512] (4q x 32 feats)
      leaky:  relu((1-a)*P) (ACT)  +  a*P  (DVE)  ->  G fp16
      mm2: block-diag(W2 x4) [128,128], 8 accumulating matmuls per
           supertile sum the 8 slots of each block in PSUM -> block sums
      gated scan:  state = gate*state + block  (gate=0 at each node's
           first block) -> Z fp16; value at a node's last block = its sum
      dma-transpose Z into token-major Zt; dma_gather (SBUF source,
           transpose mode) picks each node's last-block column ->
           per-node sums feat-major; scale by 1/deg; + b2*has
      node MLP: mm3 [104,32] (ones row carries b3), leaky, mm4 [33,7].
"""

import sys

sys.path.insert(0, "/opt/trn_rl_repo")

from contextlib import ExitStack  # noqa: E402

import numpy as np  # noqa: E402

import concourse.bass as bass  # noqa: E402
import concourse.bacc as bacc  # noqa: E402
import concourse.tile as tile  # noqa: E402
from concourse import mybir  # noqa: E402
from concourse.bass_utils import run_bass_kernel_spmd  # noqa: E402

ALU = mybir.AluOpType
ACTF = mybir.ActivationFunctionType
DT = mybir.dt

N_NODES = 100000
N_GRAPHS = 256
F_NODE = 7
F_EDGE = 6
F_GRAPH = 64
HIDDEN = 32
ALPHA = 0.01

NCORES = 8
NQ = 4
B = 8           # slots per block
CT = 512        # columns per matmul tile
SUP = 4096      # columns per supertile (8 tiles)
GCH = 512       # max idxs per dma_gather (HW crashes above ~512)
FIN = F_NODE + F_EDGE + 1  # 14: x, ea, valid

# leaky path: "lrelu" = single ACT Lrelu op; "split" = ACT relu((1-a)x) + DVE a*x add
LEAKY_MODE = "split"


def _ceil_to(a, m):
    return -(-a // m) * m


# --------------------------------------------------------------------------
# host-side preprocessing: sharding, sorting, padding, layout
# --------------------------------------------------------------------------

def prepare(x, edge_index, edge_attr, u, batch, W1, b1, W2, b2, W3, b3, W4, b4,
            n_nodes=N_NODES, ncores=NCORES):
    x = np.asarray(x, np.float32)
    ea = np.asarray(edge_attr, np.float32)
    u = np.asarray(u, np.float32)
    batch = np.asarray(batch, np.int64)
    row = np.asarray(edge_index[0], np.int64)
    col = np.asarray(edge_index[1], np.int64)
    W1 = np.asarray(W1, np.float32); b1 = np.asarray(b1, np.float32)
    W2 = np.asarray(W2, np.float32); b2 = np.asarray(b2, np.float32)
    W3 = np.asarray(W3, np.float32); b3 = np.asarray(b3, np.float32)
    W4 = np.asarray(W4, np.float32); b4 = np.asarray(b4, np.float32)

    NPC = n_nodes // ncores          # nodes per core
    NPQ = NPC // NQ                  # nodes per quarter

    order = np.argsort(col, kind="stable")
    col_s = col[order]
    row_s = row[order]
    ea_s = ea[order]

    bounds = np.searchsorted(col_s, np.arange(0, n_nodes + 1, NPQ))

    groups = []
    max_slots = 0
    for g in range(ncores * NQ):
        lo, hi = int(bounds[g]), int(bounds[g + 1])
        node0 = g * NPQ
        deg = np.bincount(col_s[lo:hi] - node0, minlength=NPQ).astype(np.int64)
        nb = -(-deg // B)
        bend = np.cumsum(nb)
        groups.append((lo, hi, deg, nb, bend))
        max_slots = max(max_slots, int(B * bend[-1]))

    LQ = _ceil_to(max(max_slots, SUP), SUP)
    NB = LQ // B
    assert NB <= 32767, f"block table too large for int16 gather: {NB}"
    NIDX = _ceil_to(NPQ, 128)
    ACOLS = _ceil_to(2 * NPQ, CT)

    meta = dict(LQ=LQ, NB=NB, NPQ=NPQ, NPC=NPC, NIDX=NIDX, ACOLS=ACOLS)

    # weights (shared by all cores)
    W1q = np.vstack([W1, b1[None, :]])                       # [14, 32]
    W1s = np.zeros((4 * FIN, 128), np.float32)
    W2s = np.zeros((128, 128), np.float32)
    for q in range(NQ):
        W1s[FIN * q:FIN * (q + 1), 32 * q:32 * (q + 1)] = W1q
        W2s[32 * q:32 * (q + 1), 32 * q:32 * (q + 1)] = W2
    W3e = np.vstack([b3[None, :], W3]).astype(np.float32)    # [104, 32]: ones row first
    W4e = np.vstack([W4, b4[None, :]]).astype(np.float32)    # [33, 7]
    b2s = np.tile(b2, NQ)[:, None].astype(np.float32)        # [128, 1]

    u_b = u[batch]                                           # [N, 64]

    in_maps = []
    for c in range(ncores):
        catT = np.zeros((4 * FIN, LQ), np.float32)
        gate = np.ones((NQ, NB), np.float16)
        bidx = np.zeros((128, NQ * (NIDX // 16)), np.int16)
        invb = np.zeros((128, NIDX), np.float32)
        hasb = np.zeros((128, NIDX), np.float32)
        for q in range(NQ):
            lo, hi, deg, nb, bend = groups[NQ * c + q]
            nE = hi - lo
            bstart = bend - nb
            edge_start = np.cumsum(deg) - deg
            node_loc = col_s[lo:hi] - (NQ * c + q) * NPQ
            slotpos = (B * bstart)[node_loc] + (np.arange(nE) - edge_start[node_loc])
            cat = np.zeros((LQ, FIN), np.float32)
            cat[slotpos, 0:F_NODE] = x[row_s[lo:hi]]
            cat[slotpos, F_NODE:F_NODE + F_EDGE] = ea_s[lo:hi]
            cat[slotpos, FIN - 1] = 1.0
            # supertile-internal transpose: slot j=8*(512*s+n)+r -> col 4096*s+512*r+n
            cat = cat.reshape(LQ // SUP, CT, B, FIN).transpose(0, 2, 1, 3).reshape(LQ, FIN)
            catT[FIN * q:FIN * (q + 1), :] = cat.T

            gate[q, bstart[deg > 0]] = 0.0

            eidx = np.maximum(bend - 1, 0)                   # node's last block
            bendpad = np.zeros(NIDX, np.int16)
            bendpad[:NPQ] = eidx.astype(np.int16)
            # wrap per gather chunk of <=GCH idxs (HW limit ~512/gather)
            parts = []
            for off in range(0, NIDX, GCH):
                ch = min(GCH, NIDX - off)
                parts.append(np.tile(
                    bendpad[off:off + ch].reshape(ch // 16, 16).T, (8, 1)))
            wrapped = np.concatenate(parts, axis=1)          # [128, NIDX/16]
            bidx[:, q * (NIDX // 16):(q + 1) * (NIDX // 16)] = wrapped
            inv = np.where(deg > 0, 1.0 / np.maximum(deg, 1), 0.0).astype(np.float32)
            has = (deg > 0).astype(np.float32)
            invb[32 * q:32 * (q + 1), :NPQ] = inv[None, :]
            hasb[32 * q:32 * (q + 1), :NPQ] = has[None, :]

        gateb = np.repeat(gate, 32, axis=0)                  # [128, NB]

        n0 = c * NPC
        in_maps.append(dict(
            catT=catT.astype(np.float16),
            gate=gateb,
            bidx=bidx,
            invb=invb,
            hasb=hasb,
            ubT=np.ascontiguousarray(u_b[n0:n0 + NPC].T.astype(np.float32)),
            xT=np.ascontiguousarray(x[n0:n0 + NPC].T.astype(np.float32)),
            W1s=W1s.astype(np.float16), W2s=W2s.astype(np.float16),
            W3e=W3e, W4e=W4e, b2s=b2s,
        ))
    return in_maps, meta


# --------------------------------------------------------------------------
# device program
# --------------------------------------------------------------------------

def build_program(meta, leaky_mode=LEAKY_MODE, num_devices=NCORES):
    LQ, NB = meta["LQ"], meta["NB"]
    NPQ, NPC = meta["NPQ"], meta["NPC"]
    NIDX, ACOLS = meta["NIDX"], meta["ACOLS"]
    S = LQ // SUP
    NT = ACOLS // CT
    NIW = NIDX // 16

    nc = bacc.Bacc("TRN2", target_bir_lowering=False, debug=False,
                   num_devices=num_devices)

    catT = nc.dram_tensor("catT", [4 * FIN, LQ], DT.float16, kind="ExternalInput").ap()
    gate = nc.dram_tensor("gate", [128, NB], DT.float16, kind="ExternalInput").ap()
    bidx = nc.dram_tensor("bidx", [128, NQ * NIW], DT.int16, kind="ExternalInput").ap()
    invb = nc.dram_tensor("invb", [128, NIDX], DT.float32, kind="ExternalInput").ap()
    hasb = nc.dram_tensor("hasb", [128, NIDX], DT.float32, kind="ExternalInput").ap()
    ubT = nc.dram_tensor("ubT", [F_GRAPH, NPC], DT.float32, kind="ExternalInput").ap()
    xT = nc.dram_tensor("xT", [F_NODE, NPC], DT.float32, kind="ExternalInput").ap()
    W1s = nc.dram_tensor("W1s", [4 * FIN, 128], DT.float16, kind="ExternalInput").ap()
    W2s = nc.dram_tensor("W2s", [128, 128], DT.float16, kind="ExternalInput").ap()
    W3e = nc.dram_tensor("W3e", [104, HIDDEN], DT.float32, kind="ExternalInput").ap()
    W4e = nc.dram_tensor("W4e", [HIDDEN + 1, F_NODE], DT.float32, kind="ExternalInput").ap()
    b2s = nc.dram_tensor("b2s", [128, 1], DT.float32, kind="ExternalInput").ap()
    outT = nc.dram_tensor("outT", [F_NODE, NPC], DT.float32, kind="ExternalOutput").ap()

    with tile.TileContext(nc) as tc, ExitStack() as ctx:
        const = ctx.enter_context(tc.tile_pool(name="const", bufs=1))
        big = ctx.enter_context(tc.tile_pool(name="big", bufs=1))
        cat_pool = ctx.enter_context(tc.tile_pool(name="cat", bufs=2))
        gate_pool = ctx.enter_context(tc.tile_pool(name="gatep", bufs=2))
        g_pool = ctx.enter_context(tc.tile_pool(name="g", bufs=12))
        z_pool = ctx.enter_context(tc.tile_pool(name="z", bufs=2))
        p1_pool = ctx.enter_context(tc.tile_pool(name="p1", bufs=3, space="PSUM"))
        p2_pool = ctx.enter_context(tc.tile_pool(name="p2", bufs=2, space="PSUM"))
        tail_ps = ctx.enter_context(tc.tile_pool(name="tailps", bufs=3, space="PSUM"))
        h3_pool = ctx.enter_context(tc.tile_pool(name="h3", bufs=3))
        tmp_pool = ctx.enter_context(tc.tile_pool(name="tmp", bufs=4))
        gout_pool = ctx.enter_context(tc.tile_pool(name="gout", bufs=2))
        asm_pool = ctx.enter_context(tc.tile_pool(name="asm", bufs=1))

        def load_const(ap, shape, dtype):
            t = const.tile(list(shape), dtype, tag=ap.tensor.name)
            nc.sync.dma_start(t[:], ap)
            return t

        W1s_sb = load_const(W1s, [4 * FIN, 128], DT.float16)
        W2s_sb = load_const(W2s, [128, 128], DT.float16)
        W3e_sb = load_const(W3e, [104, HIDDEN], DT.float32)
        W4e_sb = load_const(W4e, [HIDDEN + 1, F_NODE], DT.float32)
        b2s_sb = load_const(b2s, [128, 1], DT.float32)
        invb_sb = load_const(invb, [128, NIDX], DT.float32)
        hasb_sb = load_const(hasb, [128, NIDX], DT.float32)
        bidx_sb = load_const(bidx, [128, NQ * NIW], DT.int16)

        Zt = big.tile([128, NB], DT.float16, tag="Zt")

        def leaky(out_ap, psum_ap, shp):
            if leaky_mode == "lrelu":
                nc.scalar.activation(out_ap, psum_ap, ACTF.Lrelu,
                                     bias=0.0, scale=1.0, alpha=ALPHA)
            else:
                r = tmp_pool.tile(list(shp), DT.float32, tag="relu_tmp")
                nc.scalar.activation(r[:], psum_ap, ACTF.Relu,
                                     bias=0.0, scale=1.0 - ALPHA)
                nc.vector.scalar_tensor_tensor(
                    out=out_ap, in0=psum_ap, scalar=ALPHA, in1=r[:],
                    op0=ALU.mult, op1=ALU.add)

        # ---------------- edge phase ----------------
        z_prev = None
        for s in range(S):
            ct = cat_pool.tile([4 * FIN, SUP], DT.float16)
            nc.sync.dma_start(ct[:], catT[:, s * SUP:(s + 1) * SUP])
            gt = gate_pool.tile([128, CT], DT.float16)
            nc.sync.dma_start(gt[:], gate[:, s * CT:(s + 1) * CT])
            gs = []
            for r in range(B):
                P1 = p1_pool.tile([128, CT], DT.float32)
                nc.tensor.matmul(P1[:], lhsT=W1s_sb[:],
                                 rhs=ct[:, r * CT:(r + 1) * CT],
                                 start=True, stop=True)
                G = g_pool.tile([128, CT], DT.float16)
                leaky(G[:], P1[:], [128, CT])
                gs.append(G)
            P2 = p2_pool.tile([128, CT], DT.float32)
            for r in range(B):
                nc.tensor.matmul(P2[:], lhsT=W2s_sb[:], rhs=gs[r][:],
                                 start=(r == 0), stop=(r == B - 1))
            Z = z_pool.tile([128, CT], DT.float16)
            nc.vector.tensor_tensor_scan(
                out=Z[:], data0=gt[:], data1=P2[:],
                initial=0.0 if s == 0 else z_prev[:, CT - 1:CT],
                op0=ALU.mult, op1=ALU.add)
            z_prev = Z
            for k in range(CT // 128):
                nc.sync.dma_start_transpose(
                    Zt[:, (s * CT + k * 128):(s * CT + (k + 1) * 128)],
                    Z[:, k * 128:(k + 1) * 128])

        # ---------------- aggregate phase ----------------
        from concourse import library_config
        nc.gpsimd.load_library(library_config.mlp)
        meanb = big.tile([128, NIDX], DT.float32, tag="meanb")
        for q in range(NQ):
            p0 = 32 * q
            for off in range(0, NIDX, GCH):
                ch = min(GCH, NIDX - off)
                gq = gout_pool.tile([128, 1, GCH], DT.float16, tag="gq")
                nc.gpsimd.dma_gather(
                    out_ap=gq[:, :, 0:ch], in_ap=Zt[:],
                    idxs_ap=bidx_sb[:, q * NIW + off // 16:
                                    q * NIW + (off + ch) // 16],
                    num_idxs=ch, num_idxs_reg=ch, elem_size=128,
                    transpose=True,
                    sbuf_tokens_per_rank=128,
                    sbuf_free_dim_per_rank=256,
                    sbuf_free_dim_pad_per_rank=0,
                    sbuf_byte_offset=0)
                # mean = sums * inv(deg) + b2 * has   (inv is 0 for deg==0)
                nc.vector.tensor_tensor(out=meanb[p0:p0 + 32, off:off + ch],
                                        in0=gq[p0:p0 + 32, 0, 0:ch],
                                        in1=invb_sb[p0:p0 + 32, off:off + ch],
                                        op=ALU.mult)
                nc.vector.scalar_tensor_tensor(
                    out=meanb[p0:p0 + 32, off:off + ch],
                    in0=hasb_sb[p0:p0 + 32, off:off + ch],
                    scalar=b2s_sb[p0:p0 + 32, 0:1],
                    in1=meanb[p0:p0 + 32, off:off + ch],
                    op0=ALU.mult, op1=ALU.add)

        # ---------------- node phase ----------------
        for h in range(2):
            # asm rows: [ones(0), x(1:8), mean(8:40), u(40:104)]
            asm = asm_pool.tile([104, ACOLS], DT.float32, tag="asm")
            if ACOLS > 2 * NPQ:
                nc.vector.memset(asm[:, 2 * NPQ:ACOLS], 0.0)
            nc.vector.memset(asm[0:1, 0:ACOLS], 1.0)
            nc.sync.dma_start(asm[1:1 + F_NODE, 0:2 * NPQ],
                              xT[:, h * 2 * NPQ:(h + 1) * 2 * NPQ])
            nc.sync.dma_start(asm[40:104, 0:2 * NPQ],
                              ubT[:, h * 2 * NPQ:(h + 1) * 2 * NPQ])
            for qq in range(2):
                q = 2 * h + qq
                nc.sync.dma_start(asm[8:40, qq * NPQ:(qq + 1) * NPQ],
                                  meanb[32 * q:32 * (q + 1), 0:NPQ])
            for j in range(NT):
                P3 = tail_ps.tile([HIDDEN, CT], DT.float32, tag="tailp")
                nc.tensor.matmul(P3[:], lhsT=W3e_sb[:],
                                 rhs=asm[:, j * CT:(j + 1) * CT],
                                 start=True, stop=True)
                H3 = h3_pool.tile([HIDDEN + 1, CT], DT.float32)
                leaky(H3[0:HIDDEN, :], P3[:], [HIDDEN, CT])
                nc.vector.memset(H3[HIDDEN:HIDDEN + 1, :], 1.0)
                P4 = tail_ps.tile([F_NODE, CT], DT.float32, tag="tailp")
                nc.tensor.matmul(P4[:], lhsT=W4e_sb[:], rhs=H3[:],
                                 start=True, stop=True)
                nc.vector.tensor_copy(asm[0:F_NODE, j * CT:(j + 1) * CT], P4[:])
            nc.sync.dma_start(outT[:, h * 2 * NPQ:(h + 1) * 2 * NPQ],
                              asm[0:F_NODE, 0:2 * NPQ])

    nc.compile()
    return nc


# --------------------------------------------------------------------------
# entry point
# --------------------------------------------------------------------------

def kernel(**inputs) -> np.ndarray:
    in_maps, meta = prepare(**inputs)
    nc = build_program(meta)
    res = run_bass_kernel_spmd(nc, in_maps, list(range(NCORES)))
    NPC = meta["NPC"]
    out = np.empty((N_NODES, F_NODE), np.float32)
    for c in range(NCORES):
        out[c * NPC:(c + 1) * NPC] = np.asarray(res.results[c]["outT"],
                                                np.float32).T
    return out
